# revision 27
# baseline (speedup 1.0000x reference)
"""EventSNNFlowNetLite Bass kernel (per-core program) + host-side packing.

Sharding: 8 cores = 4 images x 2 vertical halves; each core computes its
half with redundant halo rows (no inter-core communication).

All arithmetic is plain fp32 and (modulo LSB-level summation-order changes
that empirically flip no spikes) matches the jax reference bit-for-bit.
float32r matmuls were tried and rejected NOT for the matmul itself but
because the walrus verifier forces f32r-rounded DVE writes of continuous
values (skip-adds), which flips near-threshold spikes; with the tiny
output norm a single flip exceeds the 2e-2 budget.

Optimizations vs the original baseline:
- d1 conv (60% of PE rows): the two row-taps (a=0/1) of each 2x2 phase
  stencil are K-stacked into one K=64 matmul. A row-shifted copy of the
  d2 spike plane (d2s[c,r,:] = d2[c,r+1,:]) sits at partitions 32-63 of
  d2's own tile so one flat AP spans both halves. One SBUF->SBUF DMA per
  timestep refreshes it. Halves d1's matmul rows (133k -> 66.5k per t).
- temporal means stored as exact integer spike accumulators (acc += s,
  one DVE op instead of two); the 1/(t+1) normalization is pre-folded
  into per-timestep scaled skip weights on the host.
- every matmul row-chunk gets its own PSUM bank (tile col 0); walrus
  left-over from the f32r experiment, perf-neutral for fp32.

Layout (per-partition fp32 elems; budget ~53248):
  tA [128,9750]: s1@0-31 | d1scr[36,258]@32-63 | s2 0:2442, d3 2442:4686,
      me2(acc) 4686:6930 @64-127
  tC [128,8580]: me1(acc)@0-31 | scr1[32,32,258]@32-63 | xsl@64-113
  tF [128,8580]: d2@0-31 | d2s[32,65,130]@32-63 | scr2@64-95 | scr3@96-127
  tHs[128,1636]: s3 0:612 | w_d3s 612:1636
  tW [128,2624]: floscr + packed weights
  tB [128,9750]: m1@0-31 | md1b0[32,34,258]@32-63 | m2 0:2442, md3
      2442:4686 @64-127
  tG [128,8772]: md2@0-31 | md1b1[32,32,258]@32-63 | md1b2@64-95 |
      md1b3@96-127
  tHm[128,612]: m3

d1 bands: band g covers phase-k in [k0,k1) = [0,17),[17,33),[33,49),[49,65),
local row = i0 + 2k - off, off = [0,34,66,98]; every LIF op for band g has
mem (md1b[g]) and spike target (scr) at the SAME partition base (32/32/64/96)
so walrus's same-base rule for two-SBUF-input ops holds. Band 0's spike
target is d1scr rows 2:36 directly. At t=7 bands 1-3 are staged into d1scr
via SBUF->SBUF DMA with a 2-row carry copy, and the flow conv consumes
d1scr; flow rows for band g are [32g, 32g+32).

Local buffer geometry (a = 128*h, rows are global-row windows):
  s1/m1  [32,75,130]  rows [a/2-7, a/2+68)   me1 [32,66,130] rows [a/2-1, a/2+65)
  s2/m2  [64,37,66]   rows [a/4-3, a/4+34)   me2 [64,34,66]  rows [a/4-1, a/4+33)
  s3/m3  [128,18,34]  rows [a/8-1, a/8+17)
  d3/md3 [64,34,66]   rows [a/4-1, a/4+33)
  d2/md2 [32,66,130]  rows [a/2-1, a/2+65)
  md1    bands of rows [a-1, a+129) split 34/32/32/32
"""
import numpy as np
import concourse.bass as bass
import concourse.mybir as mybir
from concourse.tile import TileContext

F32 = mybir.dt.float32
T_STEPS = 8
ALU = mybir.AluOpType


def MM(nc, out, lhsT, rhs, **kw):
    return nc.tensor.matmul(out, lhsT, rhs, **kw)

DBG_NAMES = ('s1', 's2', 's3', 'd3', 'd2', 'me1', 'me2', 'm1', 'm2')


# ------------------------------------------------------------- host packing

def phase_stencils(w):
    """w: [O, I, 3, 3] -> dict[(pr, pc, a, b)] = [I, O] combined stencils."""
    rows = {(0, 0): [0], (0, 1): [1, 2], (1, 0): [0, 1], (1, 1): [2]}
    out = {}
    for pr in range(2):
        for pc in range(2):
            for a in range(2):
                for b in range(2):
                    acc = np.zeros(w.shape[:2], np.float32)
                    for ky in rows[(pr, a)]:
                        for kx in rows[(pc, b)]:
                            acc = acc + w[:, :, ky, kx]
                    out[(pr, pc, a, b)] = np.ascontiguousarray(acc.T)
    return out


def pack_weights(inputs):
    w = {}
    w['w_e1m'] = np.ascontiguousarray(
        np.asarray(inputs['w_e1']).reshape(32, 50).T).astype(np.float32)  # [50,32]
    for nm, key in (('w_e2t', 'w_e2'), ('w_e3t', 'w_e3')):
        ww = np.asarray(inputs[key])
        I = ww.shape[1]
        t = np.stack([np.ascontiguousarray(ww[:, :, ky, kx].T)
                      for ky in range(3) for kx in range(3)])  # [9, I, O]
        w[nm] = np.ascontiguousarray(t.transpose(1, 0, 2)).reshape(I, -1)
    for nm, key in (('w_d3s', 'w_d3'),):
        S = phase_stencils(np.asarray(inputs[key]))
        I = S[(0, 0, 0, 0)].shape[0]
        t = np.stack([S[(pr, pc, a, b)] for pr in range(2) for pc in range(2)
                      for a in range(2) for b in range(2)])  # [16, I, O]
        w[nm] = np.ascontiguousarray(t.transpose(1, 0, 2)).reshape(I, -1)
    # d2: a-taps K-stacked like d1 -- [128, 8*32]
    S2 = phase_stencils(np.asarray(inputs['w_d2']))
    cols2 = []
    for pr in range(2):
        for pc in range(2):
            for b in range(2):
                cols2.append(np.concatenate(
                    [S2[(pr, pc, 0, b)], S2[(pr, pc, 1, b)]], axis=0))  # [128,32]
    w['w_d2p'] = np.ascontiguousarray(np.concatenate(cols2, axis=1)).astype(np.float32)
    # d1: a-taps K-stacked -- [64, 8*32]: rows 0-31 = S[pr,pc,0,b], rows
    # 32-63 = S[pr,pc,1,b] (the upper half multiplies the row-shifted d2 copy)
    S1 = phase_stencils(np.asarray(inputs['w_d1']))
    cols = []
    for pr in range(2):
        for pc in range(2):
            for b in range(2):
                cols.append(np.concatenate(
                    [S1[(pr, pc, 0, b)], S1[(pr, pc, 1, b)]], axis=0))  # [64,32]
    w['w_d1p'] = np.ascontiguousarray(np.concatenate(cols, axis=1)).astype(np.float32)
    sk2 = np.asarray(inputs['w_skip2'])[:, :, 0, 0].T  # [64,64]
    sk1 = np.asarray(inputs['w_skip1'])[:, :, 0, 0].T  # [32,32]
    # acc (unnormalized spike sums) replaces the running mean; scale skip
    # weights by 1/(t+1) per timestep instead
    w['w_sk2'] = np.ascontiguousarray(np.concatenate(
        [sk2 / (t + 1.0) for t in range(T_STEPS)], axis=1)).astype(np.float32)
    w['w_sk1'] = np.ascontiguousarray(np.concatenate(
        [sk1 / (t + 1.0) for t in range(T_STEPS)], axis=1)).astype(np.float32)
    wf = np.asarray(inputs['w_flow']) * 16.0
    t = np.stack([np.ascontiguousarray(wf[:, :, ky, kx].T)
                  for ky in range(3) for kx in range(3)])  # [9, 32, 2]
    w['w_flt'] = np.ascontiguousarray(t.transpose(1, 0, 2)).reshape(32, 18).astype(np.float32)
    return w


def pack_x_core(x_img, a):
    """x_img [T,2,256,256] -> [T, 2(chunks), 50, 40*128] fp32 im2col halves."""
    T = x_img.shape[0]
    xp = np.zeros((T, 2, 153, 260), np.float32)
    r0, r1 = a - 16, a + 137
    sr0, sr1 = max(r0, 0), min(r1, 256)
    xp[:, :, sr0 - r0:sr1 - r0, 2:258] = np.asarray(x_img)[:, :, sr0:sr1, :]
    full = np.empty((T, 50, 75, 128), np.float32)
    for c in range(2):
        for ky in range(5):
            for kx in range(5):
                full[:, c * 25 + ky * 5 + kx] = xp[:, c, ky:ky + 149:2, kx:kx + 255:2]
    out = np.zeros((T, 2, 50, 40, 128), np.float32)
    out[:, 0] = full[:, :, 0:40]
    out[:, 1, :, 0:35] = full[:, :, 40:75]
    return out.reshape(T, 2, 50, 40 * 128)


# ------------------------------------------------------------- device kernel

def build_kernel(repeats=1, debug=False):
    nc = bass.Bass("TRN2", target_bir_lowering=False, debug=False, num_devices=8)
    xd = nc.dram_tensor("x_e1", [T_STEPS, 2, 50, 40 * 128], F32, kind="ExternalInput").ap()
    wd = {}
    for nm, shape in (
        ('w_e1m', [50, 32]), ('w_e2t', [32, 9 * 64]), ('w_e3t', [64, 9 * 128]),
        ('w_d3s', [128, 16 * 64]), ('w_d2p', [128, 8 * 32]), ('w_d1p', [64, 8 * 32]),
        ('w_sk2', [64, 8 * 64]), ('w_sk1', [32, 8 * 32]), ('w_flt', [32, 18]),
    ):
        wd[nm] = nc.dram_tensor(nm, shape, F32, kind="ExternalInput").ap()
    flow_d = nc.dram_tensor("flow", [128, 2, 256], F32, kind="ExternalOutput").ap()
    dbg_d = {}
    if debug:
        for nm, shape in (('s1', [32, 75 * 130]), ('s2', [64, 37 * 66]),
                          ('s3', [128, 18 * 34]), ('d3', [64, 34 * 66]),
                          ('d2', [32, 66 * 130]), ('me1', [32, 66 * 130]),
                          ('me2', [64, 34 * 66]), ('m1', [32, 75 * 130]),
                          ('m2', [64, 37 * 66])):
            dbg_d[nm] = nc.dram_tensor(f"dbg_{nm}", shape, F32, kind="ExternalOutput").ap()

    with TileContext(nc) as tc:
        with tc.tile_pool(name="big", bufs=1) as sp, \
             tc.tile_pool(name="psum", bufs=8, space="PSUM") as pp:

            # ---- mega tiles (partition-slot packed); see module docstring
            tA = sp.tile([128, 9750], F32, name="tA")
            tB = sp.tile([128, 9750], F32, name="tB")
            tC = sp.tile([128, 8580], F32, name="tC")
            tF = sp.tile([128, 8580], F32, name="tF")
            tG = sp.tile([128, 8772], F32, name="tG")
            tHs = sp.tile([128, 1636], F32, name="tHs")
            tHm = sp.tile([128, 612], F32, name="tHm")
            tW = sp.tile([128, 2880], F32, name="tW")
            tD = sp.tile([128, 2244], F32, name="tD")  # d3 pair for d2 K=128
            for t_ in (tA, tB, tC, tF, tG, tHs, tHm, tW, tD):
                nc.vector.memset(t_[:], 0.0)

            def view(tile, pb, pn, o0, R, W):
                return tile[pb:pb + pn, o0:o0 + R * W].rearrange(
                    "p (r w) -> p r w", w=W)

            s1 = view(tA, 0, 32, 0, 75, 130)
            d1scr = view(tA, 32, 32, 0, 36, 258)
            s2 = view(tA, 64, 64, 0, 37, 66)
            d3 = view(tA, 64, 64, 2442, 34, 66)
            me2 = view(tA, 64, 64, 4686, 34, 66)
            me1 = view(tC, 0, 32, 0, 66, 130)
            xsl = tC[64:114, 5120:10240] if False else tC[64:114, 0:5120]
            d2 = view(tF, 0, 32, 0, 66, 130)
            s3 = view(tHs, 0, 128, 0, 18, 34)
            w_d3s = tHs[0:128, 612:612 + 1024]
            floscr = tW[0:2, 0:1024].rearrange("p (r w) -> p r w", w=256)  # [2,4,256]
            wsl = {
                'w_e2t': tW[0:32, 1024:1024 + 576],
                'w_sk1': tW[0:32, 1600:1600 + 256],
                'w_d2p': tW[0:128, 2624:2624 + 256],
                'w_flt': tW[32:64, 0:18],
                'w_e1m': tW[64:114, 0:32],
                'w_e3t': tW[64:128, 32:32 + 1152],
                'w_sk2': tW[64:128, 1184:1184 + 512],
                'w_d1p': tW[0:64, 2368:2368 + 256],
            }
            m1 = view(tB, 0, 32, 0, 75, 130)
            m2 = view(tB, 64, 64, 0, 37, 66)
            md3 = view(tB, 64, 64, 2442, 34, 66)
            md2 = view(tG, 0, 32, 0, 66, 130)
            m3 = view(tHm, 0, 128, 0, 18, 34)
            # d1 bands: [k0,k1) in phase-k, local row = i0 + 2k - off
            d1_k0 = [0, 17, 33, 49]
            d1_k1 = [17, 33, 49, 65]
            d1_off = [0, 34, 66, 98]
            md1b = [view(tB, 32, 32, 0, 34, 258),
                    view(tG, 32, 32, 0, 32, 258),
                    view(tG, 64, 32, 0, 32, 258),
                    view(tG, 96, 32, 0, 32, 258)]
            # spike targets per band (same partition base as md1b[g]);
            # band 0 writes d1scr rows 2:36 directly
            scr0 = view(tA, 32, 32, 2 * 258, 34, 258)
            scrs = [scr0,
                    view(tC, 32, 32, 0, 32, 258),
                    view(tF, 64, 32, 0, 32, 258),
                    view(tF, 96, 32, 0, 32, 258)]
            # row-shifted d2 copy at tF partitions 32-63 (d2s[c,r,:] =
            # d2[c,r+1,:]) -> d1's two row-taps become one K=64 matmul
            d2s = view(tF, 32, 32, 0, 65, 130)
            d2pair = view(tF, 0, 64, 0, 65, 130)
            # d3 pair: lower 64 partitions = d3 rows r, upper = d3 rows r+1
            d3lo = view(tD, 0, 64, 0, 34, 66)
            d3hi = view(tD, 64, 64, 0, 33, 66)
            d3pair = view(tD, 0, 128, 0, 34, 66)

            nc.gpsimd.dma_start(out=w_d3s, in_=wd['w_d3s'][:])
            for nm, ap in wsl.items():
                nc.gpsimd.dma_start(out=ap, in_=wd[nm][:])

            taps9 = [(ky, kx) for ky in range(3) for kx in range(3)]

            def enc_layer(src, dst, mem, wt, R_out, IC_out, C_out, trow, last_mem):
                nr_max = 512 // IC_out
                ng = 128 // C_out
                blocks = []
                q0 = 0
                while q0 < R_out:
                    blocks.append((q0, min(nr_max, R_out - q0)))
                    q0 += nr_max
                for rs in range(0, len(blocks), ng):
                    rnd = blocks[rs:rs + ng]
                    views = []
                    for j, (q0, nr) in enumerate(rnd):
                        psj = pp.tile([128, 512], F32, name="ps", tag="ps")
                        views.append(psj[0:C_out, :nr * IC_out]
                                     .rearrange("p (r w) -> p r w", w=IC_out))
                    for i, (ky, kx) in enumerate(taps9):
                        for j, (q0, nr) in enumerate(rnd):
                            rhs = src[:, 2 * q0 + ky: 2 * q0 + ky + 2 * (nr - 1) + 1: 2,
                                      kx: kx + 2 * (IC_out - 1) + 1: 2]
                            MM(nc, views[j], wt[:, i * C_out:(i + 1) * C_out],
                               rhs, start=(i == 0), stop=(i == 8),
                               tile_position=(trow, 0))
                    for j, (q0, nr) in enumerate(rnd):
                        nc.vector.scalar_tensor_tensor(
                            out=mem[:, q0:q0 + nr, 1:1 + IC_out],
                            in0=mem[:, q0:q0 + nr, 1:1 + IC_out], scalar=0.5,
                            in1=views[j], op0=ALU.mult, op1=ALU.add)
                    uq0 = rnd[0][0]
                    uqn = rnd[-1][0] + rnd[-1][1] - uq0
                    nc.vector.tensor_scalar(
                        out=dst[:, uq0:uq0 + uqn, 1:1 + IC_out],
                        in0=mem[:, uq0:uq0 + uqn, 1:1 + IC_out], scalar1=1.0,
                        scalar2=None, op0=ALU.is_gt)
                    if not last_mem:
                        nc.vector.tensor_tensor(
                            out=mem[:, uq0:uq0 + uqn, 1:1 + IC_out],
                            in0=mem[:, uq0:uq0 + uqn, 1:1 + IC_out],
                            in1=dst[:, uq0:uq0 + uqn, 1:1 + IC_out], op=ALU.subtract)

            def dec_layer(src, dst, mem, wt, n_k, half, C_out, trow, skip_wt=None,
                          skip_src=None, skip_trow=0, last_mem=False,
                          pair_wt=None):
                """phase conv on up2(src); dst/mem row i0+2k; src row k+a.
                pair_wt: K-stacked a-taps; src must be a [128,...] pair view
                (lower half rows r, upper half rows r+1), K=128, 2 b-taps."""
                nr_max = 512 // half
                ng = 128 // C_out
                for pr in range(2):
                    i0 = 1 - pr
                    for pc in range(2):
                        blocks = []
                        k0 = 0
                        while k0 < n_k:
                            blocks.append((k0, min(nr_max, n_k - k0)))
                            k0 += nr_max
                        for rs in range(0, len(blocks), ng):
                            rnd = blocks[rs:rs + ng]
                            views = [pp.tile([128, 512], F32, name="ps", tag="ps")
                                     [0:C_out, :nr * half]
                                     .rearrange("p (r w) -> p r w", w=half)
                                     for j, (k0, nr) in enumerate(rnd)]
                            if pair_wt is not None:
                                for idx, b in enumerate((0, 1)):
                                    wslice = pair_wt[
                                        :, ((pr * 2 + pc) * 2 + b) * C_out:
                                        ((pr * 2 + pc) * 2 + b + 1) * C_out]
                                    for j, (k0, nr) in enumerate(rnd):
                                        rhs = src[:, k0: k0 + nr,
                                                  b + pc: b + pc + half]
                                        MM(nc, views[j], wslice, rhs,
                                           start=(idx == 0), stop=(idx == 1),
                                           tile_position=(0, 0))
                            else:
                              for idx, (a, b) in enumerate(
                                    ((0, 0), (0, 1), (1, 0), (1, 1))):
                                wslice = wt[:, (((pr * 2 + pc) * 2 + a) * 2 + b) * C_out:
                                            (((pr * 2 + pc) * 2 + a) * 2 + b + 1) * C_out]
                                for j, (k0, nr) in enumerate(rnd):
                                    rhs = src[:, k0 + a: k0 + a + nr,
                                              b + pc: b + pc + half]
                                    MM(nc, views[j], wslice, rhs,
                                       start=(idx == 0), stop=(idx == 3),
                                       tile_position=(trow, 0))
                            for j, (k0, nr) in enumerate(rnd):
                                rows = slice(i0 + 2 * k0, i0 + 2 * (k0 + nr - 1) + 1, 2)
                                cols = slice(1 + pc, 1 + pc + 2 * (half - 1) + 1, 2)
                                nc.vector.scalar_tensor_tensor(
                                    out=mem[:, rows, cols],
                                    in0=mem[:, rows, cols],
                                    scalar=0.5, in1=views[j],
                                    op0=ALU.mult, op1=ALU.add)
                        # per-phase union spike + reset
                        ucols = slice(1 + pc, 1 + pc + 2 * (half - 1) + 1, 2)
                        urows = slice(i0, i0 + 2 * (n_k - 1) + 1, 2)
                        nc.vector.tensor_scalar(
                            out=dst[:, urows, ucols],
                            in0=mem[:, urows, ucols],
                            scalar1=1.0, scalar2=None, op0=ALU.is_gt)
                        if not last_mem:
                            nc.vector.tensor_tensor(
                                out=mem[:, urows, ucols],
                                in0=mem[:, urows, ucols],
                                in1=dst[:, urows, ucols], op=ALU.subtract)
                        if skip_wt is not None:
                            for rs in range(0, len(blocks), ng):
                                rnd = blocks[rs:rs + ng]
                                for j, (k0, nr) in enumerate(rnd):
                                    ps2 = pp.tile([128, 512], F32, name="ps2", tag="ps")
                                    v2 = ps2[0:C_out, :nr * half]\
                                        .rearrange("p (r w) -> p r w", w=half)
                                    srows = slice(i0 + 2 * k0,
                                                  i0 + 2 * (k0 + nr - 1) + 1, 2)
                                    MM(nc, v2, skip_wt,
                                       skip_src[:, srows, ucols],
                                       start=True, stop=True,
                                       tile_position=(skip_trow, 0))
                                    rows = slice(i0 + 2 * k0,
                                                 i0 + 2 * (k0 + nr - 1) + 1, 2)
                                    nc.vector.tensor_tensor(
                                        out=dst[:, rows, ucols],
                                        in0=dst[:, rows, ucols],
                                        in1=v2, op=ALU.add)

            for rep in range(repeats):
                for t in range(T_STEPS):
                    last = (t == T_STEPS - 1) and (rep == repeats - 1)
                    # ---- e1 in two x-chunks (rows 0-39, 40-74)
                    for ch, (cr0, crn) in enumerate(((0, 40), (40, 35))):
                        nc.sync.dma_start(out=xsl, in_=xd[t, ch])
                        xv = xsl.rearrange("p (r w) -> p r w", w=128)
                        blocks = []
                        r0 = 0
                        while r0 < crn:
                            blocks.append((r0, min(4, crn - r0)))
                            r0 += 4
                        for rs in range(0, len(blocks), 4):
                            rnd = blocks[rs:rs + 4]
                            views = [pp.tile([128, 512], F32, name="ps", tag="ps")
                                     [0:32, :nr * 128].rearrange(
                                "p (r w) -> p r w", w=128)
                                for j, (r0, nr) in enumerate(rnd)]
                            for j, (r0, nr) in enumerate(rnd):
                                MM(nc, views[j], wsl['w_e1m'],
                                   xv[:, r0:r0 + nr, :],
                                   start=True, stop=True,
                                   tile_position=(64, 0))
                            for j, (r0, nr) in enumerate(rnd):
                                gr = cr0 + r0
                                nc.vector.scalar_tensor_tensor(
                                    out=m1[:, gr:gr + nr, 1:129],
                                    in0=m1[:, gr:gr + nr, 1:129], scalar=0.5,
                                    in1=views[j], op0=ALU.mult, op1=ALU.add)
                            ug0 = cr0 + rnd[0][0]
                            ugn = rnd[-1][0] + rnd[-1][1] - rnd[0][0]
                            nc.vector.tensor_scalar(
                                out=s1[:, ug0:ug0 + ugn, 1:129],
                                in0=m1[:, ug0:ug0 + ugn, 1:129], scalar1=1.0,
                                scalar2=None, op0=ALU.is_gt)
                            nc.vector.tensor_tensor(
                                out=m1[:, ug0:ug0 + ugn, 1:129],
                                in0=m1[:, ug0:ug0 + ugn, 1:129],
                                in1=s1[:, ug0:ug0 + ugn, 1:129], op=ALU.subtract)

                    # ---- e2: s1 -> s2 (K=32)
                    enc_layer(s1, s2, m2, wsl['w_e2t'], 37, 64, 64, 0, last)
                    # ---- e3: s2 -> s3 (K=64 row base 64)
                    enc_layer(s2, s3, m3, wsl['w_e3t'], 18, 32, 128, 64, last)

                    # ---- temporal spike accumulators (exact integers; the
                    # 1/(t+1) mean normalization lives in the skip weights)
                    for me, act, off in ((me1, s1, 6), (me2, s2, 2)):
                        R = me.shape[1]
                        nc.vector.tensor_tensor(
                            out=me[:, :, :], in0=me[:, :, :],
                            in1=act[:, off:off + R, :], op=ALU.add)

                    # ---- d3: up2(s3) conv + skip2(acc2). K=128.
                    dec_layer(s3, d3, md3, w_d3s, 17, 32, 64, 0,
                              skip_wt=wsl['w_sk2'][:, t * 64:(t + 1) * 64],
                              skip_src=me2, skip_trow=64,
                              last_mem=last)
                    # ---- d2: up2(d3) conv + skip1(acc1). a-taps K-stacked
                    # (K=128) over the d3 pair copy.
                    nc.sync.dma_start(
                        out=d3lo.rearrange("p r w -> p (r w)"),
                        in_=d3.rearrange("p r w -> p (r w)"))
                    nc.sync.dma_start(
                        out=d3hi.rearrange("p r w -> p (r w)"),
                        in_=d3[:, 1:34, :].rearrange("p r w -> p (r w)"))
                    dec_layer(d3pair, d2, md2, None, 33, 64, 32, 64,
                              skip_wt=wsl['w_sk1'][:, t * 32:(t + 1) * 32],
                              skip_src=me1, skip_trow=0,
                              last_mem=last, pair_wt=wsl['w_d2p'])
                    nc.sync.dma_start(
                        out=d2s.rearrange("p r w -> p (r w)"),
                        in_=d2[:, 1:66, :].rearrange("p r w -> p (r w)"))

                    # ---- d1: up2(d2) conv, 4 bands; a-taps K-stacked (K=64)
                    for g in range(4):
                        mem_g = md1b[g]
                        scr_g = scrs[g]
                        off = d1_off[g]
                        for pr in range(2):
                            i0 = 1 - pr
                            for pc in range(2):
                                blocks = []
                                k0 = d1_k0[g]
                                while k0 < d1_k1[g]:
                                    blocks.append((k0, min(4, d1_k1[g] - k0)))
                                    k0 += 4
                                for rs in range(0, len(blocks), 4):
                                    rnd = blocks[rs:rs + 4]
                                    views = [pp.tile([128, 512], F32, name="ps",
                                                     tag="ps")
                                             [0:32, :nr * 128]
                                             .rearrange("p (r w) -> p r w", w=128)
                                             for j, (k0, nr) in enumerate(rnd)]
                                    for idx, bb in enumerate((0, 1)):
                                        wslice = wsl['w_d1p'][
                                            :, ((pr * 2 + pc) * 2 + bb) * 32:
                                            ((pr * 2 + pc) * 2 + bb + 1) * 32]
                                        for j, (k0, nr) in enumerate(rnd):
                                            rhs = d2pair[:, k0: k0 + nr,
                                                         bb + pc: bb + pc + 128]
                                            MM(nc,
                                               views[j], wslice, rhs,
                                               start=(idx == 0), stop=(idx == 1),
                                               tile_position=(0, 0))
                                    cols = slice(1 + pc, 1 + pc + 2 * 127 + 1, 2)
                                    for j, (k0, nr) in enumerate(rnd):
                                        lr0 = i0 + 2 * k0 - off
                                        mrows = slice(lr0, lr0 + 2 * (nr - 1) + 1, 2)
                                        nc.vector.scalar_tensor_tensor(
                                            out=mem_g[:, mrows, cols],
                                            in0=mem_g[:, mrows, cols], scalar=0.5,
                                            in1=views[j], op0=ALU.mult, op1=ALU.add)
                                    uk = rnd[-1][0] + rnd[-1][1] - rnd[0][0]
                                    ur0 = i0 + 2 * rnd[0][0] - off
                                    umr = slice(ur0, ur0 + 2 * (uk - 1) + 1, 2)
                                    nc.vector.tensor_scalar(
                                        out=scr_g[:, umr, cols],
                                        in0=mem_g[:, umr, cols], scalar1=1.0,
                                        scalar2=None, op0=ALU.is_gt)
                                    if not last:
                                        nc.vector.tensor_tensor(
                                            out=mem_g[:, umr, cols],
                                            in0=mem_g[:, umr, cols],
                                            in1=scr_g[:, umr, cols], op=ALU.subtract)
                        if last:
                            if g > 0:
                                # carry: last 2 d1-local rows of previous band
                                c0 = 34 if g == 1 else 32
                                nc.vector.tensor_copy(
                                    d1scr[:, 0:2, :],
                                    d1scr[:, c0:c0 + 2, :])
                                # stage band g spikes into d1scr rows 2..34
                                nc.sync.dma_start(
                                    out=d1scr[:, 2:34, :].rearrange(
                                        "p r w -> p (r w)"),
                                    in_=scr_g[:, :, :].rearrange(
                                        "p r w -> p (r w)"))
                            # flow rows for band g: f in [32g, 32g+32).
                            # staging row for d1-local row f+ky:
                            #   g=0: f+ky+2; g>0: f+ky-32g (carry rows 0,1)
                            blocks = []
                            f = 32 * g
                            while f < 32 * g + 32:
                                blocks.append((f, 2))
                                f += 2
                            for rs in range(0, len(blocks), 4):
                                rnd = blocks[rs:rs + 4]
                                views = [pp.tile([128, 512], F32, name="ps", tag="ps")
                                         [0:2, :nr * 256].rearrange(
                                    "p (r w) -> p r w", w=256)
                                    for j, (f, nr) in enumerate(rnd)]
                                for i, (ky, kx) in enumerate(taps9):
                                    for j, (f, nr) in enumerate(rnd):
                                        sr = (f + ky + 2) if g == 0 else (f + ky - 32 * g)
                                        rhs = d1scr[:, sr:sr + nr, kx:kx + 256]
                                        MM(nc,
                                           views[j], wsl['w_flt'][:, i * 2:(i + 1) * 2],
                                           rhs, start=(i == 0), stop=(i == 8),
                                           tile_position=(32, 0))
                                for j, (f, nr) in enumerate(rnd):
                                    nc.vector.tensor_copy(
                                        floscr[:, 0:nr, :], views[j])
                                    nc.sync.dma_start(
                                        out=flow_d[f:f + nr].rearrange("r p w -> p r w"),
                                        in_=floscr[0:2, 0:nr, :])
                    if debug and rep == 0 and t == DBG_STEP:
                        for nm, v in (('s1', s1), ('s2', s2), ('s3', s3),
                                      ('d3', d3), ('d2', d2), ('me1', me1),
                                      ('me2', me2), ('m1', m1), ('m2', m2)):
                            nc.sync.dma_start(
                                out=dbg_d[nm][:],
                                in_=v.rearrange("p r w -> p (r w)"))
    return nc


DBG_STEP = 1


def make_in_maps(inputs):
    wpack = pack_weights(inputs)
    x = np.asarray(inputs['x'], np.float32)
    maps = []
    for core in range(8):
        n, h = core // 2, core % 2
        m = dict(wpack)
        m['x_e1'] = pack_x_core(x[n], 128 * h)
        maps.append(m)
    return maps


def assemble(results):
    out = np.zeros((4, 2, 256, 256), np.float32)
    for core in range(8):
        n, h = core // 2, core % 2
        out[n, :, 128 * h:128 * h + 128, :] = \
            results[core]["flow"].reshape(128, 2, 256).transpose(1, 0, 2)
    return out


# ---------------------------------------------------------------- entry point

def _split_waits(nc, max_waits=1):
    """Walrus here only fits one sem-wait slot per instruction; hoist excess
    waits onto same-engine no-ops inserted right before the instruction."""
    fn = nc.m.functions[0]
    n_new = 0
    for bb in fn.blocks:
        out = []
        for inst in bb.instructions:
            si = inst.sync_info
            if si is not None and si.on_wait and len(si.on_wait) > max_waits:
                waits = list(si.on_wait)
                keep = waits[-max_waits:]
                extra = waits[:-max_waits]
                for i in range(0, len(extra), max_waits):
                    chunk = extra[i:i + max_waits]
                    nop = mybir.InstNoOp(
                        name=nc.get_next_instruction_name(),
                        sync_info=mybir.SyncInfo(on_wait=list(chunk), on_update=[]),
                        bass_nofuse=True, engine=inst.engine, text_hint="waitfix")
                    nc.register_instruction(nop)
                    out.append(nop)
                    n_new += 1
                si.on_wait = keep
            out.append(inst)
        bb.instructions = out
    return n_new


_CACHED = {}


def kernel(**inputs):
    """Full-input entry: shards across 8 NeuronCores internally."""
    from concourse.bass_utils import run_bass_kernel_spmd
    if 'nc' not in _CACHED:
        nc = build_kernel(repeats=1, debug=False)
        _split_waits(nc, max_waits=1)
        _CACHED['nc'] = nc
    nc = _CACHED['nc']
    in_maps = make_in_maps(inputs)
    res = run_bass_kernel_spmd(nc, in_maps, list(range(8)))
    return assemble(res.results)
